# revision 24
# baseline (speedup 1.0000x reference)
# Multi-head attention (B=2, S=2048, E=1024, H=16) on 8 TRN2 NeuronCores.
#
# Sharding: data-parallel over the 2 batches x tensor-parallel over 4 head
# groups (4 heads each).  Core c handles batch c//4, heads 4*(c%4)..4*(c%4)+3.
# Each core computes its heads' Q/K/V projections, attention, and a partial
# o_proj over its value features; the host sums the 4 partials per batch.
#
# Device-side layout:
#  - All matmul inputs are consumed in transposed form (contraction dim on
#    partitions); the host pre-transposes x and the weight shards.
#  - Every DRAM tensor is PARTITION-MAJOR ([128, ...] with each partition's
#    data contiguous) so DMA descriptors are 2KB+ and the prologue streams
#    at full HBM rate.
#  - Masked keys are compacted away on the host: only kept tokens (plus zero
#    padding up to KT_LEN) participate in K/V.  Padding slots get an additive
#    -30000 bias so exp() underflows to exactly 0.
#  - Logits are built transposed ([k, q]); the softmax denominator falls out
#    of the AV matmul via an extra all-ones column appended to V.
#
# Schedule:
#  - The Activation engine's exp() stream is the attention-phase floor
#    (72 x [128,1024] exp instrs, ~1.12us each).  Everything else fits
#    under that cadence:
#  - QK^T is ROW-TILED: a head only occupies 64 of the 128 contraction
#    rows, so the even head of a pair runs at tile_position (0,0) and the
#    odd head at (64,0) CONCURRENTLY in the PE array.  One attention step
#    computes logits for both heads of a pair over one 512-query chunk in
#    a single 2-bank PSUM tile -> one exp instr.
#  - bf16 everywhere off the PSUM accumulators (ex, V, valsT, w_o) keeps
#    both the PE streaming rate and the LDWEIGHTS cost down.
#  - Attention starts as soon as K-proj + the first q-chunk are done
#    (~20us): V tiles and the remaining Q chunks are interleaved into the
#    early passes as Tensor-engine fillers, o_proj for earlier token tiles
#    into the later passes.  Only the last 4 token tiles' o_proj remains
#    as a tail.
#  - The AV pair is emitted one step LATE (software pipelining) so the
#    exp-critical QKT pair of the next step always precedes it.
#  - PSUM budget: lg pool 2x[128,1024] (4 banks) + vals/sumexp pool
#    3x[65,512] (3 banks) + filler pool 1x[128,512] (1 bank) = 8 banks.

import numpy as np

KT_LEN = 1152  # default compacted+padded key extent (9 tiles of 128)
B, S, E = 2, 2048, 1024
HEADS_PER_CORE = 4
D = 64
N_CORES = 8
ET = E // 128  # 8 contraction tiles for projections
QTILES = S // 512  # 4 query chunks of 512
TT = S // 128  # 16 token tiles of 128

_compiled_nc = {}


def _build_bass(kt_len=KT_LEN):
    import concourse.mybir as mybir
    import concourse.tile as tile
    from concourse import bacc

    f32 = mybir.dt.float32
    f32r = mybir.dt.float32r
    bf16 = mybir.dt.bfloat16
    Exp = mybir.ActivationFunctionType.Exp
    KT_LEN = kt_len
    KT = KT_LEN // 128
    HPC = HEADS_PER_CORE

    nc = bacc.Bacc(None, target_bir_lowering=False, debug=False)

    # partition-major: every [128, N] row is contiguous per partition
    xp_d = nc.dram_tensor("xp", [128, QTILES * ET * 512], bf16, kind="ExternalInput")
    # wk and xkv fused per contraction-tile pair: one DMA per slab feeds
    # the K projection (fewer serialized DMA issues in the critical head)
    kxp_d = nc.dram_tensor(
        "kxp", [128, ET * (256 + KT_LEN)], bf16, kind="ExternalInput"
    )
    wqp_d = nc.dram_tensor("wqp", [128, ET * 256], bf16, kind="ExternalInput")
    wvp_d = nc.dram_tensor("wvp", [128, ET * 256], bf16, kind="ExternalInput")
    wop_d = nc.dram_tensor("wop", [128, 2 * E], bf16, kind="ExternalInput")
    mb_d = nc.dram_tensor("mbias", [KT_LEN], f32, kind="ExternalInput")
    # per-norm sumexp scratch rows for the DRAM-round-trip broadcast
    sescr_d = nc.dram_tensor("sescr", [16, 512], mybir.dt.float32r, kind="Internal")
    out_d = nc.dram_tensor("out", [S, E], bf16, kind="ExternalOutput")

    xp_v = xp_d[:].rearrange("p (c a t) -> p c a t", c=QTILES, a=ET)
    kxp_v = kxp_d[:].rearrange("p (a t) -> p a t", a=ET)
    wqp_v = wqp_d[:].rearrange("p (a d) -> p a d", a=ET)
    wvp_v = wvp_d[:].rearrange("p (a d) -> p a d", a=ET)
    wop_v = wop_d[:].rearrange("p (a e) -> p a e", a=2)
    mb_v = mb_d[:].rearrange("(k p) -> p k", p=128)  # [128, KT]

    with tile.TileContext(nc) as tc:
        with (
            tc.tile_pool(name="singles", bufs=1) as singles,
            tc.tile_pool(name="expool", bufs=4) as expool,
            tc.tile_pool(name="small", bufs=3) as small,
            tc.tile_pool(name="outst", bufs=4) as outst,
            # PSUM: 8 banks total, statically reserved:
            #   lgp  = 2 x [128,1024] (2 banks each) -> 4 banks
            #   valsp= 3 x [65,512]   (1 bank each)  -> 3 banks
            #   opp  = 1 x [128,512]  (1 bank)       -> 1 bank
            tc.tile_pool(name="lgp", bufs=2, space="PSUM") as lgp,
            tc.tile_pool(name="valsp", bufs=3, space="PSUM") as valsp,
            tc.tile_pool(name="opp", bufs=1, space="PSUM") as opp,
        ):
            wq_sb = singles.tile([128, ET, 256], bf16)
            wv_sb = singles.tile([128, ET, 256], bf16)
            wo_sb = singles.tile([128, 2, E], bf16)
            # fused wk+xkv in 4 slabs of 2 contraction tiles each, so the K
            # projection starts as soon as the first slab lands.
            kx_s = [
                singles.tile([128, 1, 256 + KT_LEN], bf16, name=f"kx{j}")
                for j in range(ET)
            ]
            mb_sb = singles.tile([128, KT], f32)
            qT_sb = singles.tile([128, 2, S // 2], bf16)
            # q-chunks 2/3 land in their own tensor (written by filler
            # projections inside pair-0 attention) so pair-1 reads never
            # serialize against unrelated writes.
            qT2_sb = singles.tile([128, 2, S // 2], bf16)
            # kT holds head-pair bl at partitions [0:64] (head 2bl) and
            # [64:128] (head 2bl+1) -- exactly the row-tiled QKT layout, so
            # no zero-fill is needed.
            kT_sb = singles.tile([128, 2, KT_LEN], bf16)
            v1_sb = singles.tile([128, KT, HPC, 65], bf16)
            valsTa = singles.tile([128, S], bf16)
            valsTb = singles.tile([128, S], bf16)
            xq = [
                singles.tile([128, ET, 512], bf16, name=f"xq{qc}")
                for qc in range(QTILES)
            ]

            def xkv_et(et):
                return kx_s[et][:, 0, 256:]

            def wk_et(et):
                return kx_s[et][:, 0, 0:256]

            # ---- DMA prologue, in consumption order (sync queue = FIFO).
            nc.gpsimd.dma_start(mb_sb, mb_v)
            for j in range(ET):
                nc.sync.dma_start(kx_s[j], kxp_v[:, j : j + 1])
            nc.sync.dma_start(wv_sb, wvp_v)
            nc.sync.dma_start(wq_sb, wqp_v)
            for qc in range(QTILES):
                nc.sync.dma_start(xq[qc], xp_v[:, qc])
            nc.sync.dma_start(wo_sb, wop_v)

            # ---- constants (off the critical DMA+PE path)
            ones_sb = singles.tile([128, 1], f32)
            nc.vector.memset(ones_sb, 1.0)
            ones64 = singles.tile([65, 64], f32r)
            nc.scalar.copy(
                ones64[64:65, :], ones_sb[64:65, 0:1].to_broadcast([1, 64])
            )
            nc.scalar.copy(
                v1_sb[:, :, :, 64:65],
                ones_sb.to_broadcast([128, KT, HPC, 1]),
            )
            # Preload the exp activation table while the DMA prologue
            # streams (otherwise the ~1.3us ACT_TABLE_LOAD lands right
            # before the first real exp, on the attention critical path).
            warm_sb = singles.tile([1, 1], f32r)
            nc.scalar.activation(warm_sb, ones_sb[0:1, 0:1], Exp, scale=0.0)

            # ---- K^T projection: [256 d, KT_LEN] in 3 chunks x 2 blocks,
            # all 6 groups open across PSUM banks; accumulation runs in 4
            # et-stages chasing the xkv slab DMAs.
            nch = (KT_LEN + 511) // 512
            base = KT_LEN // nch // 128 * 128
            KCH = []
            t0 = 0
            for ci in range(nch):
                tw = KT_LEN - t0 if ci == nch - 1 else base
                KCH.append((t0, tw))
                t0 += tw
            groups = [(bl, t0, tw) for bl in range(2) for t0, tw in KCH]
            assert len(groups) <= 7

            pskL = lgp.tile([128, 1024], f32, tag="lg", name="pskL")
            homes = []
            for gi, (bl, t0, tw) in enumerate(groups):
                if gi == 0:
                    homes.append(pskL[:, 0:tw])
                elif gi == 1:
                    homes.append(pskL[:, 512 : 512 + tw])
                elif gi < 5:
                    homes.append(
                        valsp.tile([128, tw], f32, tag="vals", name=f"pskv_{gi}")
                    )
                else:
                    homes.append(
                        opp.tile([128, tw], f32, tag="op", name=f"psko_{gi}")
                    )
            for et in range(ET):
                for gi, (bl, t0, tw) in enumerate(groups):
                    if True:
                        for ch in range(2):
                            nc.tensor.matmul(
                                homes[gi][64 * ch : 64 * (ch + 1), :],
                                lhsT=wk_et(et)[
                                    :, bl * 128 + 64 * ch : bl * 128 + 64 * (ch + 1)
                                ],
                                rhs=xkv_et(et)[:, t0 : t0 + tw],
                                start=(et == 0),
                                stop=(et == ET - 1),
                            )
            for gi, (bl, t0, tw) in enumerate(groups):
                nc.vector.tensor_copy(kT_sb[:, bl, t0 : t0 + tw], homes[gi])

            # ---- Q projection for one (q-chunk, head-pair) [128,512] block.
            # Pre-attention (lgp home) for (qc0, bl0); everything else runs
            # as two 4-et filler halves in the opp bank during attention.
            def q_dst(qc, bl):
                if qc < 2:
                    return qT_sb[:, bl, qc * 512 : (qc + 1) * 512]
                return qT2_sb[:, bl, (qc - 2) * 512 : (qc - 1) * 512]

            def q_chunk_pre(qc, bl):
                psq = lgp.tile([128, 1024], f32, tag="lg", name=f"psq_{qc}_{bl}")
                for et in range(ET):
                    for ch in range(2):
                        nc.tensor.matmul(
                            psq[64 * ch : 64 * (ch + 1), 0:512],
                            lhsT=wq_sb[
                                :, et, bl * 128 + 64 * ch : bl * 128 + 64 * (ch + 1)
                            ],
                            rhs=xq[qc][:, et],
                            start=(et == 0),
                            stop=(et == ET - 1),
                        )
                nc.vector.tensor_copy(q_dst(qc, bl), psq[:, 0:512])

            qfill_state = {}

            def emit_q_filler(qc, bl, half):
                if half == 0:
                    qfill_state["t"] = opp.tile(
                        [128, 512], f32, tag="op", name=f"psq2_{qc}_{bl}"
                    )
                t = qfill_state["t"]
                for e4 in range(4):
                    et = half * 4 + e4
                    nc.tensor.matmul(
                        t,
                        lhsT=wq_sb[:, et, bl * 128 : (bl + 1) * 128],
                        rhs=xq[qc][:, et],
                        start=(et == 0),
                        stop=(et == ET - 1),
                    )
                if half == 1:
                    nc.vector.tensor_copy(q_dst(qc, bl), t)

            # ---- V projection for one token tile: [128 t, 256 d] ----------
            # In-pass inserts alternate between the lg pool and the filler
            # bank so two V tiles can be in flight and the lg/exp rotation
            # only stalls half as often.
            def emit_v_tile(vt, use_opp=False):
                if use_opp:
                    psv = opp.tile([128, 512], f32, tag="op", name=f"psv_{vt}")
                else:
                    psv = lgp.tile([128, 1024], f32, tag="lg", name=f"psv_{vt}")
                for et in range(ET):
                    for ch in range(2):
                        nc.tensor.matmul(
                            psv[64 * ch : 64 * (ch + 1), :256],
                            lhsT=xkv_et(et)[
                                :, vt * 128 + 64 * ch : vt * 128 + 64 * (ch + 1)
                            ],
                            rhs=wv_sb[:, et],
                            start=(et == 0),
                            stop=(et == ET - 1),
                        )
                nc.vector.tensor_copy(
                    v1_sb[:, vt, :, 0:64],
                    psv[:, :256].rearrange("p (h d) -> p h d", h=HPC),
                )

            # ---- o_proj for one token tile (both 512-halves, fat store) ---
            # During attention the PSUM halves drain on DVE; the tail path
            # (ACT idle by then) splits them across Scalar and Vector.
            def emit_op_tile(ttn, tail=False):
                ot = outst.tile([128, 1024], bf16, tag="ot", name=f"ot_{ttn}")
                for ntn in range(2):
                    op = opp.tile([128, 512], f32, tag="op", name=f"op_{ttn}_{ntn}")
                    for stg, vT in ((0, valsTa), (1, valsTb)):
                        nc.tensor.matmul(
                            op,
                            lhsT=vT[:, ttn * 128 : (ttn + 1) * 128],
                            rhs=wo_sb[:, stg, ntn * 512 : (ntn + 1) * 512],
                            start=(stg == 0),
                            stop=(stg == 1),
                        )
                    nc.vector.tensor_copy(ot[:, ntn * 512 : (ntn + 1) * 512], op)
                nc.gpsimd.dma_start(out_d[ttn * 128 : (ttn + 1) * 128, :], ot)

            # ---- softmax-normalize a pass's accumulated values ------------
            # One pass covers both heads of pair bl for one 512-query chunk.
            # The sumexp row (partition 64 of each AV accumulator) is
            # broadcast across 64 partitions with a K=1 matmul (the Q7
            # partition_broadcast ucode mishandles base-partition-64 APs,
            # and DMA rejects stride-0 partition reads, so the PE stays the
            # only correct broadcast path).  The head-even result must move
            # to partitions 64:128 of valsT; engines can't shift partitions,
            # so it detours through an SBUF tile and a GpSimd-issued DMA.
            se_row = [0]

            def emit_norm_pre(p, bl, xi, valsE, valsO):
                """Evict the AV accumulators and launch the sumexp DRAM
                round-trip broadcasts (the direct broadcast paths don't
                exist: partition_broadcast's Q7 ucode mishandles
                base-partition-64 APs, and DMA rejects stride-0 SBUF
                partition reads).  The DVE-side normalize is deferred to
                emit_norm_post so the ~3us round-trip latency never
                head-of-line-blocks the DVE queue."""
                uvs = []
                for h, vals in ((2 * bl, valsE), (2 * bl + 1, valsO)):
                    uv = small.tile([65, 512], f32r, tag="uv", name=f"uv_{p}_{h}_{xi}")
                    nc.vector.tensor_copy(uv, vals)
                    row = se_row[0]
                    se_row[0] += 1
                    nc.sync.dma_start(sescr_d[row : row + 1, :], uv[64:65, :])
                    seb = small.tile(
                        [64, 512], f32r, tag="seb", name=f"seb_{p}_{h}_{xi}"
                    )
                    nc.sync.dma_start(
                        seb, sescr_d[row : row + 1, :].to_broadcast([64, 512])
                    )
                    uvs.append((h, uv, seb))
                return (p, bl, xi, uvs)

            def emit_norm_post(state):
                p, bl, xi, uvs = state
                vT = valsTa if bl == 0 else valsTb
                qoff = p * 1024 + xi * 512
                for h, uv, seb in uvs:
                    rb = small.tile([64, 512], f32, tag="rb", name=f"rb_{p}_{h}_{xi}")
                    nc.vector.reciprocal_approx_fast(rb, seb.bitcast(f32))
                    if h % 2 == 1:
                        nc.vector.tensor_mul(
                            vT[0:64, qoff : qoff + 512], uv[0:64, :], rb
                        )
                    else:
                        vn = small.tile(
                            [64, 512], bf16, tag="vn", bufs=2, name=f"vn_{p}_{h}_{xi}"
                        )
                        nc.vector.tensor_mul(vn, uv[0:64, :], rb)
                        nc.gpsimd.dma_start(vT[64:128, qoff : qoff + 512], vn)

            def emit_norm_fast(p, bl, xi, valsE, valsO):
                """Tail-critical final normalize: lower-latency PE K=1
                broadcast into the freed filler bank."""
                vT = valsTa if bl == 0 else valsTb
                qoff = p * 1024 + xi * 512
                uvs = []
                for j, (h, vals) in enumerate(((2 * bl, valsE), (2 * bl + 1, valsO))):
                    uv = small.tile([65, 512], f32r, tag="uv", name=f"uv_{p}_{h}_{xi}")
                    # ACT is idle after the last exp: evict the two
                    # accumulators on different engines concurrently
                    if j == 0:
                        nc.vector.tensor_copy(uv, vals)
                    else:
                        nc.scalar.copy(uv, vals)
                    uvs.append((h, uv))
                for h, uv in uvs:
                    se = opp.tile([64, 512], f32, tag="op", name=f"se_{p}_{h}_{xi}")
                    nc.tensor.matmul(
                        se,
                        lhsT=ones64[64:65, :],
                        rhs=uv[64:65, :],
                        start=True,
                        stop=True,
                    )
                    rb = small.tile([64, 512], f32, tag="rb", name=f"rb_{p}_{h}_{xi}")
                    nc.vector.reciprocal_approx_fast(rb, se)
                    if h % 2 == 1:
                        nc.vector.tensor_mul(
                            vT[0:64, qoff : qoff + 512], uv[0:64, :], rb
                        )
                    else:
                        vn = small.tile(
                            [64, 512], bf16, tag="vn", bufs=2, name=f"vn_{p}_{h}_{xi}"
                        )
                        nc.vector.tensor_mul(vn, uv[0:64, :], rb)
                        nc.gpsimd.dma_start(vT[64:128, qoff : qoff + 512], vn)

            # ---- pre-attention minimum: first q-chunk + first 2 V tiles ---
            q_chunk_pre(0, 0)
            emit_v_tile(0)
            emit_v_tile(1)

            # ---- attention: (qpair, q-chunk, head-pair) passes ------------
            # Each step: 2 row-tiled QKT matmuls (concurrent in the array),
            # one [128,1024] exp covering both heads, 2 AV matmuls (emitted
            # one step late).  Fillers per pass feed upcoming passes.
            passes = []
            for p in range(2):
                for xi in range(2):
                    for bl in range(2):
                        passes.append((p, bl, xi))
            # fillers[i] = list of (kt_slot, fn) for pass i
            fillers = [[] for _ in range(8)]
            # V tiles 2..KT-1 as early inserts in pass 1 (deadline: step kt
            # needs v tile kt, inserts run ~1 step after their slot).
            for j, vt in enumerate(range(2, KT)):
                fillers[0].append((j, ("v", vt)))
            # remaining Q chunks, two 4-et halves each, ordered by need:
            # pass2 needs (qc0,bl1); pass3 (qc1,bl0); pass4 (qc1,bl1);
            # pass5 (qc2,bl0); pass6 (qc2,bl1); pass7 (qc3,bl0);
            # pass8 (qc3,bl1).
            qneed = [(0, 1), (1, 0), (1, 1), (2, 0), (2, 1), (3, 0), (3, 1)]
            qslots = [
                (0, 7), (0, 8),
                (1, 1), (1, 3), (1, 5), (1, 7),
                (2, 1), (2, 3), (2, 5), (2, 7),
                (3, 1), (3, 3), (3, 5), (3, 7),
            ]
            for ci, (qc, bl) in enumerate(qneed):
                for half in range(2):
                    pi, slot = qslots[2 * ci + half]
                    fillers[pi].append((slot, ("q", qc, bl, half)))
            # o_proj: token tiles 0..11 as fillers in pair-1 passes (their
            # vals columns are fully normalized by then); 12..15 in the tail.
            opslots = [
                (4, 1), (4, 4), (4, 7),
                (5, 1), (5, 4), (5, 7),
                (6, 1), (6, 3), (6, 5), (6, 7),
                (7, 4), (7, 8),
            ]
            for j, (pi, slot) in enumerate(opslots):
                fillers[pi].append((slot, ("op", j)))

            def run_filler(spec):
                if spec[0] == "v":
                    emit_v_tile(spec[1], use_opp=(spec[1] % 2 == 1))
                elif spec[0] == "q":
                    emit_q_filler(spec[1], spec[2], spec[3])
                else:
                    emit_op_tile(spec[1])

            pending_norm = None
            norm_state = None
            pending_av = None  # (valsE, valsO, ex, bl, kt)

            def emit_av(valsE, valsO, ex, bl, kt):
                nc.tensor.matmul(
                    valsE,
                    lhsT=v1_sb[:, kt, 2 * bl],
                    rhs=ex[:, 0:512],
                    start=(kt == 0),
                    stop=(kt == KT - 1),
                )
                nc.tensor.matmul(
                    valsO,
                    lhsT=v1_sb[:, kt, 2 * bl + 1],
                    rhs=ex[:, 512:1024],
                    start=(kt == 0),
                    stop=(kt == KT - 1),
                )

            for pi, (p, bl, xi) in enumerate(passes):
                qsrc = qT_sb if p == 0 else qT2_sb
                xs = slice(xi * 512, (xi + 1) * 512)
                pass_fill = sorted(fillers[pi])
                fi = 0
                valsE = valsO = None
                for kt in range(KT):
                    lg = lgp.tile(
                        [128, 1024], f32, tag="lg", name=f"lg_{p}_{bl}_{xi}_{kt}"
                    )
                    ks = slice(kt * 128, (kt + 1) * 128)
                    nc.tensor.matmul(
                        lg[:, 0:512],
                        lhsT=kT_sb[0:64, bl, ks],
                        rhs=qsrc[0:64, bl, xs],
                        start=True,
                        stop=True,
                    )
                    nc.tensor.matmul(
                        lg[:, 512:1024],
                        lhsT=kT_sb[64:128, bl, ks],
                        rhs=qsrc[64:128, bl, xs],
                        start=True,
                        stop=True,
                    )
                    ex = expool.tile(
                        [128, 1024], bf16, tag="ex", name=f"ex_{p}_{bl}_{xi}_{kt}"
                    )
                    nc.scalar.activation(
                        ex, lg, Exp, bias=mb_sb[:, kt : kt + 1], scale=0.125
                    )
                    # flush the previous step's AVs, then (at kt==0) the
                    # previous pass's normalize; vals tiles are allocated
                    # after it so the pool rotation frees banks in
                    # dependency order.
                    if pending_av is not None:
                        emit_av(*pending_av)
                        pending_av = None
                    if kt == 0:
                        if pending_norm is not None:
                            norm_state = emit_norm_pre(*pending_norm)
                            pending_norm = None
                        valsE = valsp.tile(
                            [65, 512], f32, tag="vals", name=f"vals_{p}_{bl}_{xi}_E"
                        )
                        valsO = valsp.tile(
                            [65, 512], f32, tag="vals", name=f"vals_{p}_{bl}_{xi}_O"
                        )
                    if kt == 3 and norm_state is not None:
                        emit_norm_post(norm_state)
                        norm_state = None
                    pending_av = (valsE, valsO, ex, bl, kt)
                    while fi < len(pass_fill) and pass_fill[fi][0] <= kt:
                        run_filler(pass_fill[fi][1])
                        fi += 1
                while fi < len(pass_fill):
                    run_filler(pass_fill[fi][1])
                    fi += 1
                pending_norm = (p, bl, xi, valsE, valsO)

            emit_av(*pending_av)
            pending_av = None
            emit_norm_fast(*pending_norm)

            # ---- o_proj tail: token tiles 12..15.  Nothing left to overlap
            # with, so spread the 8 half-tiles over all 8 freed PSUM banks
            # and drain with both the Scalar and Vector engines.
            def op_homes():
                lga = lgp.tile([128, 1024], f32, tag="lg", name="opfA")
                lgb = lgp.tile([128, 1024], f32, tag="lg", name="opfB")
                yield lga[:, 0:512]
                yield lga[:, 512:1024]
                yield lgb[:, 0:512]
                yield lgb[:, 512:1024]
                for k in range(3):
                    yield valsp.tile([128, 512], f32, tag="vals", name=f"opfv{k}")
                yield opp.tile([128, 512], f32, tag="op", name="opfo")

            homegen = op_homes()
            tail_ops = []
            tc.cur_priority += 1000000  # keep the tail behind all pass work
            # phase 1: valsTa-stage matmuls only -- they depend on the
            # PREVIOUS pass's normalize, so they run (and keep the PE warm)
            # while the final pass's norm chain and vn-DMA are in flight.
            for ttn in range(12, TT):
                for ntn in range(2):
                    op = next(homegen)
                    tail_ops.append((ttn, ntn, op))
                    nc.tensor.matmul(
                        op,
                        lhsT=valsTa[:, ttn * 128 : (ttn + 1) * 128],
                        rhs=wo_sb[:, 0, ntn * 512 : (ntn + 1) * 512],
                        start=True,
                        stop=False,
                    )
            # phase 2: valsTb accumulation + eviction + store per tile
            ots = {}
            for ttn, ntn, op in tail_ops:
                nc.tensor.matmul(
                    op,
                    lhsT=valsTb[:, ttn * 128 : (ttn + 1) * 128],
                    rhs=wo_sb[:, 1, ntn * 512 : (ntn + 1) * 512],
                    start=False,
                    stop=True,
                )
                if ntn == 0:
                    ots[ttn] = outst.tile(
                        [128, 1024], bf16, tag="ot", name=f"otf_{ttn}"
                    )
                    nc.scalar.copy(ots[ttn][:, 0:512], op)
                else:
                    nc.vector.tensor_copy(ots[ttn][:, 512:1024], op)
                    nc.sync.dma_start(
                        out_d[ttn * 128 : (ttn + 1) * 128, :], ots[ttn]
                    )

    nc.compile()
    return nc


def _get_nc(kt_len=KT_LEN):
    if kt_len not in _compiled_nc:
        _compiled_nc[kt_len] = _build_bass(kt_len)
    return _compiled_nc[kt_len]


def pick_kt_len(src_padding_mask):
    """Smallest supported compacted key extent covering every batch's kept
    tokens (KT_LEN default covers it with ~5 sigma of slack for random
    masks; anything larger falls back to a wider, slower build)."""
    need = int(np.max(np.sum(np.asarray(src_padding_mask), axis=1)))
    need = max(need, 256)
    need = (need + 127) // 128 * 128
    return KT_LEN if need <= KT_LEN else need


def make_in_maps(x, src_padding_mask, w_qkv, w_o, kt_len=None):
    """Shard the full inputs into the 8 per-core input maps (all DRAM
    tensors partition-major: [128, ...] with per-partition rows
    contiguous)."""
    import ml_dtypes

    bf16 = ml_dtypes.bfloat16
    if kt_len is None:
        kt_len = pick_kt_len(src_padding_mask)
    x = np.asarray(x, dtype=np.float32)
    mask = np.asarray(src_padding_mask)
    w_qkv = np.asarray(w_qkv, dtype=np.float32)
    w_o = np.asarray(w_o, dtype=np.float32)

    def pmaj(a2d):
        """[E, N] row-major -> [128, ET, N] partition-major."""
        e, n = a2d.shape
        return np.ascontiguousarray(
            a2d.reshape(e // 128, 128, n).transpose(1, 0, 2)
        )

    # w_qkv rows are per-head interleaved: head h -> rows [192h, 192h+192),
    # split 64/64/64 into q/k/v.
    wr = w_qkv.reshape(16, 3, D, E)  # [head, qkv, d, e]

    in_maps = []
    per_batch = {}
    for b in range(B):
        xb = x[b]  # [S, E]
        xT = xb.T  # [E, S]
        # [128, qc, a, t] so each partition's per-q-chunk slab is contiguous
        xpm = (
            xT.reshape(ET, 128, QTILES, 512)
            .transpose(1, 2, 0, 3)
            .reshape(128, -1)
        )
        idx = np.nonzero(mask[b])[0]
        nk = len(idx)
        assert nk <= kt_len, f"kept keys {nk} exceed kt_len {kt_len}"
        xkvT = np.zeros((E, kt_len), np.float32)
        xkvT[:, :nk] = xb[idx].T
        mb = np.full((kt_len,), -30000.0, np.float32)
        mb[:nk] = 0.0
        per_batch[b] = (
            np.ascontiguousarray(xpm).astype(bf16),
            pmaj(xkvT),  # [128, ET, kt_len] f32
            mb,
        )

    for c in range(N_CORES):
        b, g = divmod(c, N_CORES // B)
        xpm, xkvpm, mb = per_batch[b]
        heads = slice(g * HEADS_PER_CORE, (g + 1) * HEADS_PER_CORE)
        wq = wr[heads, 0].reshape(256, E)  # [4*64, E]
        wk = wr[heads, 1].reshape(256, E)
        wv = wr[heads, 2].reshape(256, E)
        wo = (
            w_o[:, g * 256 : (g + 1) * 256]
            .reshape(E, 2, 2, D)[:, :, ::-1, :]
            .reshape(E, 256)
            .T
        )  # [256, E]
        # fuse wk and xkv along the per-contraction-tile free axis
        kx = np.concatenate([pmaj(wk.T), xkvpm], axis=2)  # [128, ET, 256+kt]
        in_maps.append(
            {
                "xp": xpm,
                "kxp": np.ascontiguousarray(kx).reshape(128, -1).astype(bf16),
                "wqp": pmaj(wq.T).reshape(128, -1).astype(bf16),
                "wvp": pmaj(wv.T).reshape(128, -1).astype(bf16),
                "wop": pmaj(wo).reshape(128, -1).astype(bf16),
                "mbias": mb,
            }
        )
    return in_maps


def combine_outputs(outs):
    """Sum the 4 per-head-group partials for each batch."""
    full = np.zeros((B, S, E), np.float32)
    for c in range(N_CORES):
        full[c // (N_CORES // B)] += np.asarray(outs[c]).astype(np.float32)
    return full


def kernel(x, src_padding_mask, w_qkv, w_o, _trace=False):
    from concourse.bass_utils import run_bass_kernel_spmd

    kt_len = pick_kt_len(src_padding_mask)
    nc = _get_nc(kt_len)
    in_maps = make_in_maps(x, src_padding_mask, w_qkv, w_o, kt_len)
    kwargs = {}
    if _trace:
        kwargs = dict(trace=True, trace_cores=list(range(N_CORES)))
    res = run_bass_kernel_spmd(nc, in_maps, core_ids=list(range(N_CORES)), **kwargs)
    out = combine_outputs([r["out"] for r in res.results])
    if _trace:
        kernel._last_result = res
    return out


# revision 25
# speedup vs baseline: 1.0234x; 1.0234x over previous
# Multi-head attention (B=2, S=2048, E=1024, H=16) on 8 TRN2 NeuronCores.
#
# Sharding: data-parallel over the 2 batches x tensor-parallel over 4 head
# groups (4 heads each).  Core c handles batch c//4, heads 4*(c%4)..4*(c%4)+3.
# Each core computes its heads' Q/K/V projections, attention, and a partial
# o_proj over its value features; the host sums the 4 partials per batch.
#
# Device-side layout:
#  - All matmul inputs are consumed in transposed form (contraction dim on
#    partitions); the host pre-transposes x and the weight shards.
#  - Every DRAM tensor is PARTITION-MAJOR ([128, ...] with each partition's
#    data contiguous) so DMA descriptors are 2KB+ and the prologue streams
#    at full HBM rate.
#  - Masked keys are compacted away on the host: only kept tokens (plus zero
#    padding up to KT_LEN) participate in K/V.  Padding slots get an additive
#    -30000 bias so exp() underflows to exactly 0.
#  - Logits are built transposed ([k, q]); the softmax denominator falls out
#    of the AV matmul via an extra all-ones column appended to V.
#
# Schedule:
#  - The Activation engine's exp() stream is the attention-phase floor
#    (72 x [128,1024] exp instrs, ~1.12us each).  Everything else fits
#    under that cadence:
#  - QK^T is ROW-TILED: a head only occupies 64 of the 128 contraction
#    rows, so the even head of a pair runs at tile_position (0,0) and the
#    odd head at (64,0) CONCURRENTLY in the PE array.  One attention step
#    computes logits for both heads of a pair over one 512-query chunk in
#    a single 2-bank PSUM tile -> one exp instr.
#  - bf16 everywhere off the PSUM accumulators (ex, V, valsT, w_o) keeps
#    both the PE streaming rate and the LDWEIGHTS cost down.
#  - Attention starts as soon as K-proj + the first q-chunk are done
#    (~20us): V tiles and the remaining Q chunks are interleaved into the
#    early passes as Tensor-engine fillers, o_proj for earlier token tiles
#    into the later passes.  Only the last 4 token tiles' o_proj remains
#    as a tail.
#  - The AV pair is emitted one step LATE (software pipelining) so the
#    exp-critical QKT pair of the next step always precedes it.
#  - PSUM budget: lg pool 2x[128,1024] (4 banks) + vals/sumexp pool
#    3x[65,512] (3 banks) + filler pool 1x[128,512] (1 bank) = 8 banks.

import numpy as np

KT_LEN = 1152  # default compacted+padded key extent (9 tiles of 128)
B, S, E = 2, 2048, 1024
HEADS_PER_CORE = 4
D = 64
N_CORES = 8
ET = E // 128  # 8 contraction tiles for projections
QTILES = S // 512  # 4 query chunks of 512
TT = S // 128  # 16 token tiles of 128

_compiled_nc = {}


def _build_bass(kt_len=KT_LEN):
    import concourse.mybir as mybir
    import concourse.tile as tile
    from concourse import bacc

    f32 = mybir.dt.float32
    f32r = mybir.dt.float32r
    bf16 = mybir.dt.bfloat16
    Exp = mybir.ActivationFunctionType.Exp
    KT_LEN = kt_len
    KT = KT_LEN // 128
    HPC = HEADS_PER_CORE

    nc = bacc.Bacc(None, target_bir_lowering=False, debug=False)

    # partition-major: every [128, N] row is contiguous per partition
    xp_d = nc.dram_tensor("xp", [128, QTILES * ET * 512], bf16, kind="ExternalInput")
    # wk and xkv fused per contraction-tile pair: one DMA per slab feeds
    # the K projection (fewer serialized DMA issues in the critical head)
    kxp_d = nc.dram_tensor(
        "kxp", [128, ET * (256 + KT_LEN)], bf16, kind="ExternalInput"
    )
    wqp_d = nc.dram_tensor("wqp", [128, ET * 256], bf16, kind="ExternalInput")
    wvp_d = nc.dram_tensor("wvp", [128, ET * 256], bf16, kind="ExternalInput")
    wop_d = nc.dram_tensor("wop", [128, 2 * E], bf16, kind="ExternalInput")
    mb_d = nc.dram_tensor("mbias", [KT_LEN], f32, kind="ExternalInput")
    # per-norm sumexp scratch rows for the DRAM-round-trip broadcast
    sescr_d = nc.dram_tensor("sescr", [16, 512], mybir.dt.float32r, kind="Internal")
    out_d = nc.dram_tensor("out", [S, E], bf16, kind="ExternalOutput")

    xp_v = xp_d[:].rearrange("p (c a t) -> p c a t", c=QTILES, a=ET)
    kxp_v = kxp_d[:].rearrange("p (a t) -> p a t", a=ET)
    wqp_v = wqp_d[:].rearrange("p (a d) -> p a d", a=ET)
    wvp_v = wvp_d[:].rearrange("p (a d) -> p a d", a=ET)
    wop_v = wop_d[:].rearrange("p (a e) -> p a e", a=2)
    mb_v = mb_d[:].rearrange("(k p) -> p k", p=128)  # [128, KT]

    with tile.TileContext(nc) as tc:
        with (
            tc.tile_pool(name="singles", bufs=1) as singles,
            tc.tile_pool(name="expool", bufs=4) as expool,
            tc.tile_pool(name="small", bufs=3) as small,
            tc.tile_pool(name="outst", bufs=4) as outst,
            # PSUM: 8 banks total, statically reserved:
            #   lgp  = 2 x [128,1024] (2 banks each) -> 4 banks
            #   valsp= 3 x [65,512]   (1 bank each)  -> 3 banks
            #   opp  = 1 x [128,512]  (1 bank)       -> 1 bank
            tc.tile_pool(name="lgp", bufs=2, space="PSUM") as lgp,
            tc.tile_pool(name="valsp", bufs=3, space="PSUM") as valsp,
            tc.tile_pool(name="opp", bufs=1, space="PSUM") as opp,
        ):
            wq_sb = singles.tile([128, ET, 256], bf16)
            wv_sb = singles.tile([128, ET, 256], bf16)
            wo_sb = singles.tile([128, 2, E], bf16)
            # fused wk+xkv in 4 slabs of 2 contraction tiles each, so the K
            # projection starts as soon as the first slab lands.
            kx_s = [
                singles.tile([128, 2, 256 + KT_LEN], bf16, name=f"kx{j}")
                for j in range(4)
            ]
            mb_sb = singles.tile([128, KT], f32)
            qT_sb = singles.tile([128, 2, S // 2], bf16)
            # q-chunks 2/3 land in their own tensor (written by filler
            # projections inside pair-0 attention) so pair-1 reads never
            # serialize against unrelated writes.
            qT2_sb = singles.tile([128, 2, S // 2], bf16)
            # kT holds head-pair bl at partitions [0:64] (head 2bl) and
            # [64:128] (head 2bl+1) -- exactly the row-tiled QKT layout, so
            # no zero-fill is needed.
            kT_sb = singles.tile([128, 2, KT_LEN], bf16)
            v1_sb = singles.tile([128, KT, HPC, 65], bf16)
            valsTa = singles.tile([128, S], bf16)
            valsTb = singles.tile([128, S], bf16)
            xq = [
                singles.tile([128, ET, 512], bf16, name=f"xq{qc}")
                for qc in range(QTILES)
            ]

            def xkv_et(et):
                return kx_s[et // 2][:, et % 2, 256:]

            def wk_et(et):
                return kx_s[et // 2][:, et % 2, 0:256]

            # ---- DMA prologue, in consumption order (sync queue = FIFO).
            nc.gpsimd.dma_start(mb_sb, mb_v)
            for j in range(4):
                nc.sync.dma_start(kx_s[j], kxp_v[:, 2 * j : 2 * j + 2])
            nc.sync.dma_start(wv_sb, wvp_v)
            nc.sync.dma_start(wq_sb, wqp_v)
            for qc in range(QTILES):
                nc.sync.dma_start(xq[qc], xp_v[:, qc])
            nc.sync.dma_start(wo_sb, wop_v)

            # ---- constants (off the critical DMA+PE path)
            ones_sb = singles.tile([128, 1], f32)
            nc.vector.memset(ones_sb, 1.0)
            ones64 = singles.tile([65, 64], f32r)
            nc.scalar.copy(
                ones64[64:65, :], ones_sb[64:65, 0:1].to_broadcast([1, 64])
            )
            nc.scalar.copy(
                v1_sb[:, :, :, 64:65],
                ones_sb.to_broadcast([128, KT, HPC, 1]),
            )
            # Preload the exp activation table while the DMA prologue
            # streams (otherwise the ~1.3us ACT_TABLE_LOAD lands right
            # before the first real exp, on the attention critical path).
            warm_sb = singles.tile([1, 1], f32r)
            nc.scalar.activation(warm_sb, ones_sb[0:1, 0:1], Exp, scale=0.0)

            # ---- K^T projection: [256 d, KT_LEN] in 3 chunks x 2 blocks,
            # all 6 groups open across PSUM banks; accumulation runs in 4
            # et-stages chasing the xkv slab DMAs.
            nch = (KT_LEN + 511) // 512
            base = KT_LEN // nch // 128 * 128
            KCH = []
            t0 = 0
            for ci in range(nch):
                tw = KT_LEN - t0 if ci == nch - 1 else base
                KCH.append((t0, tw))
                t0 += tw
            groups = [(bl, t0, tw) for bl in range(2) for t0, tw in KCH]
            assert len(groups) <= 7

            pskL = lgp.tile([128, 1024], f32, tag="lg", name="pskL")
            homes = []
            for gi, (bl, t0, tw) in enumerate(groups):
                if gi == 0:
                    homes.append(pskL[:, 0:tw])
                elif gi == 1:
                    homes.append(pskL[:, 512 : 512 + tw])
                elif gi < 5:
                    homes.append(
                        valsp.tile([128, tw], f32, tag="vals", name=f"pskv_{gi}")
                    )
                else:
                    homes.append(
                        opp.tile([128, tw], f32, tag="op", name=f"psko_{gi}")
                    )
            for stage in range(4):
                for gi, (bl, t0, tw) in enumerate(groups):
                    for et in (2 * stage, 2 * stage + 1):
                        for ch in range(2):
                            nc.tensor.matmul(
                                homes[gi][64 * ch : 64 * (ch + 1), :],
                                lhsT=wk_et(et)[
                                    :, bl * 128 + 64 * ch : bl * 128 + 64 * (ch + 1)
                                ],
                                rhs=xkv_et(et)[:, t0 : t0 + tw],
                                start=(et == 0),
                                stop=(et == ET - 1),
                            )
            for gi, (bl, t0, tw) in enumerate(groups):
                nc.vector.tensor_copy(kT_sb[:, bl, t0 : t0 + tw], homes[gi])

            # ---- Q projection for one (q-chunk, head-pair) [128,512] block.
            # Pre-attention (lgp home) for (qc0, bl0); everything else runs
            # as two 4-et filler halves in the opp bank during attention.
            def q_dst(qc, bl):
                if qc < 2:
                    return qT_sb[:, bl, qc * 512 : (qc + 1) * 512]
                return qT2_sb[:, bl, (qc - 2) * 512 : (qc - 1) * 512]

            def q_chunk_pre(qc, bl):
                psq = lgp.tile([128, 1024], f32, tag="lg", name=f"psq_{qc}_{bl}")
                for et in range(ET):
                    for ch in range(2):
                        nc.tensor.matmul(
                            psq[64 * ch : 64 * (ch + 1), 0:512],
                            lhsT=wq_sb[
                                :, et, bl * 128 + 64 * ch : bl * 128 + 64 * (ch + 1)
                            ],
                            rhs=xq[qc][:, et],
                            start=(et == 0),
                            stop=(et == ET - 1),
                        )
                nc.vector.tensor_copy(q_dst(qc, bl), psq[:, 0:512])

            qfill_state = {}

            def emit_q_filler(qc, bl, half):
                if half == 0:
                    qfill_state["t"] = opp.tile(
                        [128, 512], f32, tag="op", name=f"psq2_{qc}_{bl}"
                    )
                t = qfill_state["t"]
                for e4 in range(4):
                    et = half * 4 + e4
                    nc.tensor.matmul(
                        t,
                        lhsT=wq_sb[:, et, bl * 128 : (bl + 1) * 128],
                        rhs=xq[qc][:, et],
                        start=(et == 0),
                        stop=(et == ET - 1),
                    )
                if half == 1:
                    nc.vector.tensor_copy(q_dst(qc, bl), t)

            # ---- V projection for one token tile: [128 t, 256 d] ----------
            # In-pass inserts alternate between the lg pool and the filler
            # bank so two V tiles can be in flight and the lg/exp rotation
            # only stalls half as often.
            def emit_v_tile(vt, use_opp=False):
                if use_opp:
                    psv = opp.tile([128, 512], f32, tag="op", name=f"psv_{vt}")
                else:
                    psv = lgp.tile([128, 1024], f32, tag="lg", name=f"psv_{vt}")
                for et in range(ET):
                    for ch in range(2):
                        nc.tensor.matmul(
                            psv[64 * ch : 64 * (ch + 1), :256],
                            lhsT=xkv_et(et)[
                                :, vt * 128 + 64 * ch : vt * 128 + 64 * (ch + 1)
                            ],
                            rhs=wv_sb[:, et],
                            start=(et == 0),
                            stop=(et == ET - 1),
                        )
                nc.vector.tensor_copy(
                    v1_sb[:, vt, :, 0:64],
                    psv[:, :256].rearrange("p (h d) -> p h d", h=HPC),
                )

            # ---- o_proj for one token tile (both 512-halves, fat store) ---
            # During attention the PSUM halves drain on DVE; the tail path
            # (ACT idle by then) splits them across Scalar and Vector.
            def emit_op_tile(ttn, tail=False):
                ot = outst.tile([128, 1024], bf16, tag="ot", name=f"ot_{ttn}")
                for ntn in range(2):
                    op = opp.tile([128, 512], f32, tag="op", name=f"op_{ttn}_{ntn}")
                    for stg, vT in ((0, valsTa), (1, valsTb)):
                        nc.tensor.matmul(
                            op,
                            lhsT=vT[:, ttn * 128 : (ttn + 1) * 128],
                            rhs=wo_sb[:, stg, ntn * 512 : (ntn + 1) * 512],
                            start=(stg == 0),
                            stop=(stg == 1),
                        )
                    nc.vector.tensor_copy(ot[:, ntn * 512 : (ntn + 1) * 512], op)
                nc.gpsimd.dma_start(out_d[ttn * 128 : (ttn + 1) * 128, :], ot)

            # ---- softmax-normalize a pass's accumulated values ------------
            # One pass covers both heads of pair bl for one 512-query chunk.
            # The sumexp row (partition 64 of each AV accumulator) is
            # broadcast across 64 partitions with a K=1 matmul (the Q7
            # partition_broadcast ucode mishandles base-partition-64 APs,
            # and DMA rejects stride-0 partition reads, so the PE stays the
            # only correct broadcast path).  The head-even result must move
            # to partitions 64:128 of valsT; engines can't shift partitions,
            # so it detours through an SBUF tile and a GpSimd-issued DMA.
            se_row = [0]

            def emit_norm_pre(p, bl, xi, valsE, valsO):
                """Evict the AV accumulators and launch the sumexp DRAM
                round-trip broadcasts (the direct broadcast paths don't
                exist: partition_broadcast's Q7 ucode mishandles
                base-partition-64 APs, and DMA rejects stride-0 SBUF
                partition reads).  The DVE-side normalize is deferred to
                emit_norm_post so the ~3us round-trip latency never
                head-of-line-blocks the DVE queue."""
                uvs = []
                for h, vals in ((2 * bl, valsE), (2 * bl + 1, valsO)):
                    uv = small.tile([65, 512], f32r, tag="uv", name=f"uv_{p}_{h}_{xi}")
                    nc.vector.tensor_copy(uv, vals)
                    row = se_row[0]
                    se_row[0] += 1
                    nc.sync.dma_start(sescr_d[row : row + 1, :], uv[64:65, :])
                    seb = small.tile(
                        [64, 512], f32r, tag="seb", name=f"seb_{p}_{h}_{xi}"
                    )
                    nc.sync.dma_start(
                        seb, sescr_d[row : row + 1, :].to_broadcast([64, 512])
                    )
                    uvs.append((h, uv, seb))
                return (p, bl, xi, uvs)

            def emit_norm_post(state):
                p, bl, xi, uvs = state
                vT = valsTa if bl == 0 else valsTb
                qoff = p * 1024 + xi * 512
                for h, uv, seb in uvs:
                    rb = small.tile([64, 512], f32, tag="rb", name=f"rb_{p}_{h}_{xi}")
                    nc.vector.reciprocal_approx_fast(rb, seb.bitcast(f32))
                    if h % 2 == 1:
                        nc.vector.tensor_mul(
                            vT[0:64, qoff : qoff + 512], uv[0:64, :], rb
                        )
                    else:
                        vn = small.tile(
                            [64, 512], bf16, tag="vn", bufs=2, name=f"vn_{p}_{h}_{xi}"
                        )
                        nc.vector.tensor_mul(vn, uv[0:64, :], rb)
                        nc.gpsimd.dma_start(vT[64:128, qoff : qoff + 512], vn)

            def emit_norm_fast(p, bl, xi, valsE, valsO):
                """Tail-critical final normalize: lower-latency PE K=1
                broadcast into the freed filler bank."""
                vT = valsTa if bl == 0 else valsTb
                qoff = p * 1024 + xi * 512
                uvs = []
                for h, vals in ((2 * bl, valsE), (2 * bl + 1, valsO)):
                    uv = small.tile([65, 512], f32r, tag="uv", name=f"uv_{p}_{h}_{xi}")
                    nc.vector.tensor_copy(uv, vals)
                    uvs.append((h, uv))
                for h, uv in uvs:
                    se = opp.tile([64, 512], f32, tag="op", name=f"se_{p}_{h}_{xi}")
                    nc.tensor.matmul(
                        se,
                        lhsT=ones64[64:65, :],
                        rhs=uv[64:65, :],
                        start=True,
                        stop=True,
                    )
                    rb = small.tile([64, 512], f32, tag="rb", name=f"rb_{p}_{h}_{xi}")
                    nc.vector.reciprocal_approx_fast(rb, se)
                    if h % 2 == 1:
                        nc.vector.tensor_mul(
                            vT[0:64, qoff : qoff + 512], uv[0:64, :], rb
                        )
                    else:
                        vn = small.tile(
                            [64, 512], bf16, tag="vn", bufs=2, name=f"vn_{p}_{h}_{xi}"
                        )
                        nc.vector.tensor_mul(vn, uv[0:64, :], rb)
                        nc.gpsimd.dma_start(vT[64:128, qoff : qoff + 512], vn)

            # ---- pre-attention minimum: first q-chunk + first 2 V tiles ---
            q_chunk_pre(0, 0)
            emit_v_tile(0)
            emit_v_tile(1)

            # ---- attention: (qpair, q-chunk, head-pair) passes ------------
            # Each step: 2 row-tiled QKT matmuls (concurrent in the array),
            # one [128,1024] exp covering both heads, 2 AV matmuls (emitted
            # one step late).  Fillers per pass feed upcoming passes.
            passes = []
            for p in range(2):
                for xi in range(2):
                    for bl in range(2):
                        passes.append((p, bl, xi))
            # fillers[i] = list of (kt_slot, fn) for pass i
            fillers = [[] for _ in range(8)]
            # V tiles 2..KT-1 as early inserts in pass 1 (deadline: step kt
            # needs v tile kt, inserts run ~1 step after their slot).
            for j, vt in enumerate(range(2, KT)):
                fillers[0].append((j, ("v", vt)))
            # remaining Q chunks, two 4-et halves each, ordered by need:
            # pass2 needs (qc0,bl1); pass3 (qc1,bl0); pass4 (qc1,bl1);
            # pass5 (qc2,bl0); pass6 (qc2,bl1); pass7 (qc3,bl0);
            # pass8 (qc3,bl1).
            qneed = [(0, 1), (1, 0), (1, 1), (2, 0), (2, 1), (3, 0), (3, 1)]
            qslots = [
                (0, 7), (0, 8),
                (1, 1), (1, 3), (1, 5), (1, 7),
                (2, 1), (2, 3), (2, 5), (2, 7),
                (3, 1), (3, 3), (3, 5), (3, 7),
            ]
            for ci, (qc, bl) in enumerate(qneed):
                for half in range(2):
                    pi, slot = qslots[2 * ci + half]
                    fillers[pi].append((slot, ("q", qc, bl, half)))
            # o_proj: token tiles 0..11 as fillers in pair-1 passes (their
            # vals columns are fully normalized by then); 12..15 in the tail.
            opslots = [
                (4, 1), (4, 4), (4, 7),
                (5, 1), (5, 4), (5, 7),
                (6, 1), (6, 3), (6, 5), (6, 7),
                (7, 4), (7, 8),
            ]
            for j, (pi, slot) in enumerate(opslots):
                fillers[pi].append((slot, ("op", j)))

            def run_filler(spec):
                if spec[0] == "v":
                    emit_v_tile(spec[1], use_opp=(spec[1] % 2 == 1))
                elif spec[0] == "q":
                    emit_q_filler(spec[1], spec[2], spec[3])
                else:
                    emit_op_tile(spec[1])

            pending_norm = None
            norm_state = None
            pending_av = None  # (valsE, valsO, ex, bl, kt)

            def emit_av(valsE, valsO, ex, bl, kt):
                nc.tensor.matmul(
                    valsE,
                    lhsT=v1_sb[:, kt, 2 * bl],
                    rhs=ex[:, 0:512],
                    start=(kt == 0),
                    stop=(kt == KT - 1),
                )
                nc.tensor.matmul(
                    valsO,
                    lhsT=v1_sb[:, kt, 2 * bl + 1],
                    rhs=ex[:, 512:1024],
                    start=(kt == 0),
                    stop=(kt == KT - 1),
                )

            for pi, (p, bl, xi) in enumerate(passes):
                qsrc = qT_sb if p == 0 else qT2_sb
                xs = slice(xi * 512, (xi + 1) * 512)
                pass_fill = sorted(fillers[pi])
                fi = 0
                valsE = valsO = None
                for kt in range(KT):
                    lg = lgp.tile(
                        [128, 1024], f32, tag="lg", name=f"lg_{p}_{bl}_{xi}_{kt}"
                    )
                    ks = slice(kt * 128, (kt + 1) * 128)
                    nc.tensor.matmul(
                        lg[:, 0:512],
                        lhsT=kT_sb[0:64, bl, ks],
                        rhs=qsrc[0:64, bl, xs],
                        start=True,
                        stop=True,
                    )
                    nc.tensor.matmul(
                        lg[:, 512:1024],
                        lhsT=kT_sb[64:128, bl, ks],
                        rhs=qsrc[64:128, bl, xs],
                        start=True,
                        stop=True,
                    )
                    ex = expool.tile(
                        [128, 1024], bf16, tag="ex", name=f"ex_{p}_{bl}_{xi}_{kt}"
                    )
                    nc.scalar.activation(
                        ex, lg, Exp, bias=mb_sb[:, kt : kt + 1], scale=0.125
                    )
                    # flush the previous step's AVs, then (at kt==0) the
                    # previous pass's normalize; vals tiles are allocated
                    # after it so the pool rotation frees banks in
                    # dependency order.
                    if pending_av is not None:
                        emit_av(*pending_av)
                        pending_av = None
                    if kt == 0:
                        if pending_norm is not None:
                            norm_state = emit_norm_pre(*pending_norm)
                            pending_norm = None
                        valsE = valsp.tile(
                            [65, 512], f32, tag="vals", name=f"vals_{p}_{bl}_{xi}_E"
                        )
                        valsO = valsp.tile(
                            [65, 512], f32, tag="vals", name=f"vals_{p}_{bl}_{xi}_O"
                        )
                    if kt == 3 and norm_state is not None:
                        emit_norm_post(norm_state)
                        norm_state = None
                    pending_av = (valsE, valsO, ex, bl, kt)
                    while fi < len(pass_fill) and pass_fill[fi][0] <= kt:
                        run_filler(pass_fill[fi][1])
                        fi += 1
                while fi < len(pass_fill):
                    run_filler(pass_fill[fi][1])
                    fi += 1
                pending_norm = (p, bl, xi, valsE, valsO)

            emit_av(*pending_av)
            pending_av = None
            emit_norm_fast(*pending_norm)

            # ---- o_proj tail: token tiles 12..15.  Nothing left to overlap
            # with, so spread the 8 half-tiles over all 8 freed PSUM banks
            # and drain with both the Scalar and Vector engines.
            def op_homes():
                lga = lgp.tile([128, 1024], f32, tag="lg", name="opfA")
                lgb = lgp.tile([128, 1024], f32, tag="lg", name="opfB")
                yield lga[:, 0:512]
                yield lga[:, 512:1024]
                yield lgb[:, 0:512]
                yield lgb[:, 512:1024]
                for k in range(3):
                    yield valsp.tile([128, 512], f32, tag="vals", name=f"opfv{k}")
                yield opp.tile([128, 512], f32, tag="op", name="opfo")

            homegen = op_homes()
            tail_ops = []
            tc.cur_priority += 1000000  # keep the tail behind all pass work
            # phase 1: valsTa-stage matmuls only -- they depend on the
            # PREVIOUS pass's normalize, so they run (and keep the PE warm)
            # while the final pass's norm chain and vn-DMA are in flight.
            for ttn in range(12, TT):
                for ntn in range(2):
                    op = next(homegen)
                    tail_ops.append((ttn, ntn, op))
                    nc.tensor.matmul(
                        op,
                        lhsT=valsTa[:, ttn * 128 : (ttn + 1) * 128],
                        rhs=wo_sb[:, 0, ntn * 512 : (ntn + 1) * 512],
                        start=True,
                        stop=False,
                    )
            # phase 2: valsTb accumulation + eviction + store per tile
            ots = {}
            for ttn, ntn, op in tail_ops:
                nc.tensor.matmul(
                    op,
                    lhsT=valsTb[:, ttn * 128 : (ttn + 1) * 128],
                    rhs=wo_sb[:, 1, ntn * 512 : (ntn + 1) * 512],
                    start=False,
                    stop=True,
                )
                if ntn == 0:
                    ots[ttn] = outst.tile(
                        [128, 1024], bf16, tag="ot", name=f"otf_{ttn}"
                    )
                    nc.scalar.copy(ots[ttn][:, 0:512], op)
                else:
                    nc.vector.tensor_copy(ots[ttn][:, 512:1024], op)
                    nc.sync.dma_start(
                        out_d[ttn * 128 : (ttn + 1) * 128, :], ots[ttn]
                    )

    nc.compile()
    return nc


def _get_nc(kt_len=KT_LEN):
    if kt_len not in _compiled_nc:
        _compiled_nc[kt_len] = _build_bass(kt_len)
    return _compiled_nc[kt_len]


def pick_kt_len(src_padding_mask):
    """Smallest supported compacted key extent covering every batch's kept
    tokens (KT_LEN default covers it with ~5 sigma of slack for random
    masks; anything larger falls back to a wider, slower build)."""
    need = int(np.max(np.sum(np.asarray(src_padding_mask), axis=1)))
    need = max(need, 256)
    need = (need + 127) // 128 * 128
    return KT_LEN if need <= KT_LEN else need


def make_in_maps(x, src_padding_mask, w_qkv, w_o, kt_len=None):
    """Shard the full inputs into the 8 per-core input maps (all DRAM
    tensors partition-major: [128, ...] with per-partition rows
    contiguous)."""
    import ml_dtypes

    bf16 = ml_dtypes.bfloat16
    if kt_len is None:
        kt_len = pick_kt_len(src_padding_mask)
    x = np.asarray(x, dtype=np.float32)
    mask = np.asarray(src_padding_mask)
    w_qkv = np.asarray(w_qkv, dtype=np.float32)
    w_o = np.asarray(w_o, dtype=np.float32)

    def pmaj(a2d):
        """[E, N] row-major -> [128, ET, N] partition-major."""
        e, n = a2d.shape
        return np.ascontiguousarray(
            a2d.reshape(e // 128, 128, n).transpose(1, 0, 2)
        )

    # w_qkv rows are per-head interleaved: head h -> rows [192h, 192h+192),
    # split 64/64/64 into q/k/v.
    wr = w_qkv.reshape(16, 3, D, E)  # [head, qkv, d, e]

    in_maps = []
    per_batch = {}
    for b in range(B):
        xb = x[b]  # [S, E]
        xT = xb.T  # [E, S]
        # [128, qc, a, t] so each partition's per-q-chunk slab is contiguous
        xpm = (
            xT.reshape(ET, 128, QTILES, 512)
            .transpose(1, 2, 0, 3)
            .reshape(128, -1)
        )
        idx = np.nonzero(mask[b])[0]
        nk = len(idx)
        assert nk <= kt_len, f"kept keys {nk} exceed kt_len {kt_len}"
        xkvT = np.zeros((E, kt_len), np.float32)
        xkvT[:, :nk] = xb[idx].T
        mb = np.full((kt_len,), -30000.0, np.float32)
        mb[:nk] = 0.0
        per_batch[b] = (
            np.ascontiguousarray(xpm).astype(bf16),
            pmaj(xkvT),  # [128, ET, kt_len] f32
            mb,
        )

    for c in range(N_CORES):
        b, g = divmod(c, N_CORES // B)
        xpm, xkvpm, mb = per_batch[b]
        heads = slice(g * HEADS_PER_CORE, (g + 1) * HEADS_PER_CORE)
        wq = wr[heads, 0].reshape(256, E)  # [4*64, E]
        wk = wr[heads, 1].reshape(256, E)
        wv = wr[heads, 2].reshape(256, E)
        wo = (
            w_o[:, g * 256 : (g + 1) * 256]
            .reshape(E, 2, 2, D)[:, :, ::-1, :]
            .reshape(E, 256)
            .T
        )  # [256, E]
        # fuse wk and xkv along the per-contraction-tile free axis
        kx = np.concatenate([pmaj(wk.T), xkvpm], axis=2)  # [128, ET, 256+kt]
        in_maps.append(
            {
                "xp": xpm,
                "kxp": np.ascontiguousarray(kx).reshape(128, -1).astype(bf16),
                "wqp": pmaj(wq.T).reshape(128, -1).astype(bf16),
                "wvp": pmaj(wv.T).reshape(128, -1).astype(bf16),
                "wop": pmaj(wo).reshape(128, -1).astype(bf16),
                "mbias": mb,
            }
        )
    return in_maps


def combine_outputs(outs):
    """Sum the 4 per-head-group partials for each batch."""
    full = np.zeros((B, S, E), np.float32)
    for c in range(N_CORES):
        full[c // (N_CORES // B)] += np.asarray(outs[c]).astype(np.float32)
    return full


def kernel(x, src_padding_mask, w_qkv, w_o, _trace=False):
    from concourse.bass_utils import run_bass_kernel_spmd

    kt_len = pick_kt_len(src_padding_mask)
    nc = _get_nc(kt_len)
    in_maps = make_in_maps(x, src_padding_mask, w_qkv, w_o, kt_len)
    kwargs = {}
    if _trace:
        kwargs = dict(trace=True, trace_cores=list(range(N_CORES)))
    res = run_bass_kernel_spmd(nc, in_maps, core_ids=list(range(N_CORES)), **kwargs)
    out = combine_outputs([r["out"] for r in res.results])
    if _trace:
        kernel._last_result = res
    return out


# revision 26
# speedup vs baseline: 1.0376x; 1.0139x over previous
# Multi-head attention (B=2, S=2048, E=1024, H=16) on 8 TRN2 NeuronCores.
#
# Sharding: data-parallel over the 2 batches x tensor-parallel over 4 head
# groups (4 heads each).  Core c handles batch c//4, heads 4*(c%4)..4*(c%4)+3.
# Each core computes its heads' Q/K/V projections, attention, and a partial
# o_proj over its value features; the host sums the 4 partials per batch.
#
# Device-side layout:
#  - All matmul inputs are consumed in transposed form (contraction dim on
#    partitions); the host pre-transposes x and the weight shards.
#  - Every DRAM tensor is PARTITION-MAJOR ([128, ...] with each partition's
#    data contiguous) so DMA descriptors are 2KB+ and the prologue streams
#    at full HBM rate.
#  - Masked keys are compacted away on the host: only kept tokens (plus zero
#    padding up to KT_LEN) participate in K/V.  Padding slots get an additive
#    -30000 bias so exp() underflows to exactly 0.
#  - Logits are built transposed ([k, q]); the softmax denominator falls out
#    of the AV matmul via an extra all-ones column appended to V.
#
# Schedule:
#  - The Activation engine's exp() stream is the attention-phase floor
#    (72 x [128,1024] exp instrs, ~1.12us each).  Everything else fits
#    under that cadence:
#  - QK^T is ROW-TILED: a head only occupies 64 of the 128 contraction
#    rows, so the even head of a pair runs at tile_position (0,0) and the
#    odd head at (64,0) CONCURRENTLY in the PE array.  One attention step
#    computes logits for both heads of a pair over one 512-query chunk in
#    a single 2-bank PSUM tile -> one exp instr.
#  - bf16 everywhere off the PSUM accumulators (ex, V, valsT, w_o) keeps
#    both the PE streaming rate and the LDWEIGHTS cost down.
#  - Attention starts as soon as K-proj + the first q-chunk are done
#    (~20us): V tiles and the remaining Q chunks are interleaved into the
#    early passes as Tensor-engine fillers, o_proj for earlier token tiles
#    into the later passes.  Only the last 4 token tiles' o_proj remains
#    as a tail.
#  - The AV pair is emitted one step LATE (software pipelining) so the
#    exp-critical QKT pair of the next step always precedes it.
#  - PSUM budget: lg pool 2x[128,1024] (4 banks) + vals/sumexp pool
#    3x[65,512] (3 banks) + filler pool 1x[128,512] (1 bank) = 8 banks.

import numpy as np

KT_LEN = 1152  # default compacted+padded key extent (9 tiles of 128)
B, S, E = 2, 2048, 1024
HEADS_PER_CORE = 4
D = 64
N_CORES = 8
ET = E // 128  # 8 contraction tiles for projections
QTILES = S // 512  # 4 query chunks of 512
TT = S // 128  # 16 token tiles of 128

_compiled_nc = {}


def _build_bass(kt_len=KT_LEN):
    import concourse.mybir as mybir
    import concourse.tile as tile
    from concourse import bacc

    f32 = mybir.dt.float32
    f32r = mybir.dt.float32r
    bf16 = mybir.dt.bfloat16
    Exp = mybir.ActivationFunctionType.Exp
    KT_LEN = kt_len
    KT = KT_LEN // 128
    HPC = HEADS_PER_CORE

    nc = bacc.Bacc(None, target_bir_lowering=False, debug=False)

    # partition-major: every [128, N] row is contiguous per partition
    xp_d = nc.dram_tensor("xp", [128, QTILES * ET * 512], bf16, kind="ExternalInput")
    # wk and xkv fused per contraction-tile pair: one DMA per slab feeds
    # the K projection (fewer serialized DMA issues in the critical head)
    kxp_d = nc.dram_tensor(
        "kxp", [128, ET * (256 + KT_LEN)], bf16, kind="ExternalInput"
    )
    wqp_d = nc.dram_tensor("wqp", [128, ET * 256], bf16, kind="ExternalInput")
    wvp_d = nc.dram_tensor("wvp", [128, ET * 256], bf16, kind="ExternalInput")
    wop_d = nc.dram_tensor("wop", [128, 2 * E], bf16, kind="ExternalInput")
    mb_d = nc.dram_tensor("mbias", [KT_LEN], f32, kind="ExternalInput")
    # per-norm sumexp scratch rows for the DRAM-round-trip broadcast
    sescr_d = nc.dram_tensor("sescr", [16, 512], mybir.dt.float32r, kind="Internal")
    out_d = nc.dram_tensor("out", [S, E], bf16, kind="ExternalOutput")

    xp_v = xp_d[:].rearrange("p (c a t) -> p c a t", c=QTILES, a=ET)
    kxp_v = kxp_d[:].rearrange("p (a t) -> p a t", a=ET)
    wqp_v = wqp_d[:].rearrange("p (a d) -> p a d", a=ET)
    wvp_v = wvp_d[:].rearrange("p (a d) -> p a d", a=ET)
    wop_v = wop_d[:].rearrange("p (a e) -> p a e", a=2)
    mb_v = mb_d[:].rearrange("(k p) -> p k", p=128)  # [128, KT]

    with tile.TileContext(nc) as tc:
        with (
            tc.tile_pool(name="singles", bufs=1) as singles,
            tc.tile_pool(name="expool", bufs=4) as expool,
            tc.tile_pool(name="small", bufs=3) as small,
            tc.tile_pool(name="outst", bufs=4) as outst,
            # PSUM: 8 banks total, statically reserved:
            #   lgp  = 2 x [128,1024] (2 banks each) -> 4 banks
            #   valsp= 3 x [65,512]   (1 bank each)  -> 3 banks
            #   opp  = 1 x [128,512]  (1 bank)       -> 1 bank
            tc.tile_pool(name="lgp", bufs=2, space="PSUM") as lgp,
            tc.tile_pool(name="valsp", bufs=3, space="PSUM") as valsp,
            tc.tile_pool(name="opp", bufs=1, space="PSUM") as opp,
        ):
            wq_sb = singles.tile([128, ET, 256], bf16)
            wv_sb = singles.tile([128, ET, 256], bf16)
            wo_sb = singles.tile([128, 2, E], bf16)
            # fused wk+xkv in 4 slabs of 2 contraction tiles each, so the K
            # projection starts as soon as the first slab lands.
            kx_s = [
                singles.tile([128, 2, 256 + KT_LEN], bf16, name=f"kx{j}")
                for j in range(4)
            ]
            mb_sb = singles.tile([128, KT], f32)
            qT_sb = singles.tile([128, 2, S // 2], bf16)
            # q-chunks 2/3 land in their own tensor (written by filler
            # projections inside pair-0 attention) so pair-1 reads never
            # serialize against unrelated writes.
            qT2_sb = singles.tile([128, 2, S // 2], bf16)
            # kT holds head-pair bl at partitions [0:64] (head 2bl) and
            # [64:128] (head 2bl+1) -- exactly the row-tiled QKT layout, so
            # no zero-fill is needed.
            kT_sb = singles.tile([128, 2, KT_LEN], bf16)
            v1_sb = singles.tile([128, KT, HPC, 65], bf16)
            valsTa = singles.tile([128, S], bf16)
            valsTb = singles.tile([128, S], bf16)
            xq = [
                singles.tile([128, ET, 512], bf16, name=f"xq{qc}")
                for qc in range(QTILES)
            ]

            def xkv_et(et):
                return kx_s[et // 2][:, et % 2, 256:]

            def wk_et(et):
                return kx_s[et // 2][:, et % 2, 0:256]

            # ---- DMA prologue, in consumption order (sync queue = FIFO).
            nc.gpsimd.dma_start(mb_sb, mb_v)
            for j in range(4):
                nc.sync.dma_start(kx_s[j], kxp_v[:, 2 * j : 2 * j + 2])
            nc.sync.dma_start(wv_sb, wvp_v)
            nc.sync.dma_start(wq_sb, wqp_v)
            for qc in range(QTILES):
                nc.sync.dma_start(xq[qc], xp_v[:, qc])
            nc.sync.dma_start(wo_sb, wop_v)

            # ---- constants (off the critical DMA+PE path)
            ones_sb = singles.tile([128, 1], f32)
            nc.vector.memset(ones_sb, 1.0)
            ones64 = singles.tile([65, 64], f32r)
            nc.scalar.copy(
                ones64[64:65, :], ones_sb[64:65, 0:1].to_broadcast([1, 64])
            )
            nc.scalar.copy(
                v1_sb[:, :, :, 64:65],
                ones_sb.to_broadcast([128, KT, HPC, 1]),
            )
            # Preload the exp activation table while the DMA prologue
            # streams (otherwise the ~1.3us ACT_TABLE_LOAD lands right
            # before the first real exp, on the attention critical path).
            warm_sb = singles.tile([1, 1], f32r)
            nc.scalar.activation(warm_sb, ones_sb[0:1, 0:1], Exp, scale=0.0)

            # ---- K^T projection: [256 d, KT_LEN] in 3 chunks x 2 blocks,
            # all 6 groups open across PSUM banks; accumulation runs in 4
            # et-stages chasing the xkv slab DMAs.
            nch = (KT_LEN + 511) // 512
            base = KT_LEN // nch // 128 * 128
            KCH = []
            t0 = 0
            for ci in range(nch):
                tw = KT_LEN - t0 if ci == nch - 1 else base
                KCH.append((t0, tw))
                t0 += tw
            groups = [(bl, t0, tw) for bl in range(2) for t0, tw in KCH]
            assert len(groups) <= 7

            pskL = lgp.tile([128, 1024], f32, tag="lg", name="pskL")
            homes = []
            for gi, (bl, t0, tw) in enumerate(groups):
                if gi == 0:
                    homes.append(pskL[:, 0:tw])
                elif gi == 1:
                    homes.append(pskL[:, 512 : 512 + tw])
                elif gi < 5:
                    homes.append(
                        valsp.tile([128, tw], f32, tag="vals", name=f"pskv_{gi}")
                    )
                else:
                    homes.append(
                        opp.tile([128, tw], f32, tag="op", name=f"psko_{gi}")
                    )
            for stage in range(4):
                for gi, (bl, t0, tw) in enumerate(groups):
                    for et in (2 * stage, 2 * stage + 1):
                        for ch in range(2):
                            nc.tensor.matmul(
                                homes[gi][64 * ch : 64 * (ch + 1), :],
                                lhsT=wk_et(et)[
                                    :, bl * 128 + 64 * ch : bl * 128 + 64 * (ch + 1)
                                ],
                                rhs=xkv_et(et)[:, t0 : t0 + tw],
                                start=(et == 0),
                                stop=(et == ET - 1),
                            )
            for gi, (bl, t0, tw) in enumerate(groups):
                nc.vector.tensor_copy(kT_sb[:, bl, t0 : t0 + tw], homes[gi])

            # ---- Q projection for one (q-chunk, head-pair) [128,512] block.
            # Pre-attention (lgp home) for (qc0, bl0); everything else runs
            # as two 4-et filler halves in the opp bank during attention.
            def q_dst(qc, bl):
                if qc < 2:
                    return qT_sb[:, bl, qc * 512 : (qc + 1) * 512]
                return qT2_sb[:, bl, (qc - 2) * 512 : (qc - 1) * 512]

            def q_chunk_pre(qc, bl):
                psq = lgp.tile([128, 1024], f32, tag="lg", name=f"psq_{qc}_{bl}")
                for et in range(ET):
                    for ch in range(2):
                        nc.tensor.matmul(
                            psq[64 * ch : 64 * (ch + 1), 0:512],
                            lhsT=wq_sb[
                                :, et, bl * 128 + 64 * ch : bl * 128 + 64 * (ch + 1)
                            ],
                            rhs=xq[qc][:, et],
                            start=(et == 0),
                            stop=(et == ET - 1),
                        )
                nc.vector.tensor_copy(q_dst(qc, bl), psq[:, 0:512])

            qfill_state = {}

            def emit_q_filler(qc, bl, half):
                if half == 0:
                    qfill_state["t"] = opp.tile(
                        [128, 512], f32, tag="op", name=f"psq2_{qc}_{bl}"
                    )
                t = qfill_state["t"]
                for e4 in range(4):
                    et = half * 4 + e4
                    nc.tensor.matmul(
                        t,
                        lhsT=wq_sb[:, et, bl * 128 : (bl + 1) * 128],
                        rhs=xq[qc][:, et],
                        start=(et == 0),
                        stop=(et == ET - 1),
                    )
                if half == 1:
                    nc.vector.tensor_copy(q_dst(qc, bl), t)

            # ---- V projection for one token tile: [128 t, 256 d] ----------
            # In-pass inserts alternate between the lg pool and the filler
            # bank so two V tiles can be in flight and the lg/exp rotation
            # only stalls half as often.
            def emit_v_tile(vt, use_opp=False):
                if use_opp:
                    psv = opp.tile([128, 512], f32, tag="op", name=f"psv_{vt}")
                else:
                    psv = lgp.tile([128, 1024], f32, tag="lg", name=f"psv_{vt}")
                for et in range(ET):
                    for ch in range(2):
                        nc.tensor.matmul(
                            psv[64 * ch : 64 * (ch + 1), :256],
                            lhsT=xkv_et(et)[
                                :, vt * 128 + 64 * ch : vt * 128 + 64 * (ch + 1)
                            ],
                            rhs=wv_sb[:, et],
                            start=(et == 0),
                            stop=(et == ET - 1),
                        )
                nc.vector.tensor_copy(
                    v1_sb[:, vt, :, 0:64],
                    psv[:, :256].rearrange("p (h d) -> p h d", h=HPC),
                )

            # ---- o_proj for one token tile (both 512-halves, fat store) ---
            # During attention the PSUM halves drain on DVE; the tail path
            # (ACT idle by then) splits them across Scalar and Vector.
            def emit_op_tile(ttn, tail=False):
                ot = outst.tile([128, 1024], bf16, tag="ot", name=f"ot_{ttn}")
                for ntn in range(2):
                    op = opp.tile([128, 512], f32, tag="op", name=f"op_{ttn}_{ntn}")
                    for stg, vT in ((0, valsTa), (1, valsTb)):
                        nc.tensor.matmul(
                            op,
                            lhsT=vT[:, ttn * 128 : (ttn + 1) * 128],
                            rhs=wo_sb[:, stg, ntn * 512 : (ntn + 1) * 512],
                            start=(stg == 0),
                            stop=(stg == 1),
                        )
                    nc.vector.tensor_copy(ot[:, ntn * 512 : (ntn + 1) * 512], op)
                nc.gpsimd.dma_start(out_d[ttn * 128 : (ttn + 1) * 128, :], ot)

            # ---- softmax-normalize a pass's accumulated values ------------
            # One pass covers both heads of pair bl for one 512-query chunk.
            # The sumexp row (partition 64 of each AV accumulator) is
            # broadcast across 64 partitions with a K=1 matmul (the Q7
            # partition_broadcast ucode mishandles base-partition-64 APs,
            # and DMA rejects stride-0 partition reads, so the PE stays the
            # only correct broadcast path).  The head-even result must move
            # to partitions 64:128 of valsT; engines can't shift partitions,
            # so it detours through an SBUF tile and a GpSimd-issued DMA.
            se_row = [0]

            def emit_norm_pre(p, bl, xi, valsE, valsO):
                """Evict the AV accumulators and launch the sumexp DRAM
                round-trip broadcasts (the direct broadcast paths don't
                exist: partition_broadcast's Q7 ucode mishandles
                base-partition-64 APs, and DMA rejects stride-0 SBUF
                partition reads).  The DVE-side normalize is deferred to
                emit_norm_post so the ~3us round-trip latency never
                head-of-line-blocks the DVE queue."""
                uvs = []
                for h, vals in ((2 * bl, valsE), (2 * bl + 1, valsO)):
                    uv = small.tile([65, 512], f32r, tag="uv", name=f"uv_{p}_{h}_{xi}")
                    nc.vector.tensor_copy(uv, vals)
                    row = se_row[0]
                    se_row[0] += 1
                    nc.sync.dma_start(sescr_d[row : row + 1, :], uv[64:65, :])
                    seb = small.tile(
                        [64, 512], f32r, tag="seb", name=f"seb_{p}_{h}_{xi}"
                    )
                    nc.sync.dma_start(
                        seb, sescr_d[row : row + 1, :].to_broadcast([64, 512])
                    )
                    uvs.append((h, uv, seb))
                return (p, bl, xi, uvs)

            def emit_norm_post(state):
                p, bl, xi, uvs = state
                vT = valsTa if bl == 0 else valsTb
                qoff = p * 1024 + xi * 512
                for h, uv, seb in uvs:
                    rb = small.tile([64, 512], f32, tag="rb", name=f"rb_{p}_{h}_{xi}")
                    nc.vector.reciprocal_approx_fast(rb, seb.bitcast(f32))
                    if h % 2 == 1:
                        nc.vector.tensor_mul(
                            vT[0:64, qoff : qoff + 512], uv[0:64, :], rb
                        )
                    else:
                        vn = small.tile(
                            [64, 512], bf16, tag="vn", bufs=2, name=f"vn_{p}_{h}_{xi}"
                        )
                        nc.vector.tensor_mul(vn, uv[0:64, :], rb)
                        nc.gpsimd.dma_start(vT[64:128, qoff : qoff + 512], vn)

            def emit_norm_fast(p, bl, xi, valsE, valsO):
                """Tail-critical final normalize: lower-latency PE K=1
                broadcast into the freed filler bank."""
                vT = valsTa if bl == 0 else valsTb
                qoff = p * 1024 + xi * 512
                uvs = []
                for j, (h, vals) in enumerate(((2 * bl, valsE), (2 * bl + 1, valsO))):
                    uv = small.tile([65, 512], f32r, tag="uv", name=f"uv_{p}_{h}_{xi}")
                    # ACT is idle after the last exp: evict the two
                    # accumulators on different engines concurrently
                    if j == 0:
                        nc.vector.tensor_copy(uv, vals)
                    else:
                        nc.scalar.copy(uv, vals)
                    uvs.append((h, uv))
                for h, uv in uvs:
                    se = opp.tile([64, 512], f32, tag="op", name=f"se_{p}_{h}_{xi}")
                    nc.tensor.matmul(
                        se,
                        lhsT=ones64[64:65, :],
                        rhs=uv[64:65, :],
                        start=True,
                        stop=True,
                    )
                    rb = small.tile([64, 512], f32, tag="rb", name=f"rb_{p}_{h}_{xi}")
                    nc.vector.reciprocal_approx_fast(rb, se)
                    if h % 2 == 1:
                        nc.vector.tensor_mul(
                            vT[0:64, qoff : qoff + 512], uv[0:64, :], rb
                        )
                    else:
                        vn = small.tile(
                            [64, 512], bf16, tag="vn", bufs=2, name=f"vn_{p}_{h}_{xi}"
                        )
                        nc.vector.tensor_mul(vn, uv[0:64, :], rb)
                        nc.gpsimd.dma_start(vT[64:128, qoff : qoff + 512], vn)

            # ---- pre-attention minimum: first q-chunk (both head pairs)
            # + first 2 V tiles.  (qc0,bl1) is needed at pass 2's first
            # step; as a pass-1 tail filler its eviction landed exactly on
            # the pass boundary and stalled the exp stream ~2.6us.
            q_chunk_pre(0, 0)
            q_chunk_pre(0, 1)
            emit_v_tile(0)
            emit_v_tile(1)

            # ---- attention: (qpair, q-chunk, head-pair) passes ------------
            # Each step: 2 row-tiled QKT matmuls (concurrent in the array),
            # one [128,1024] exp covering both heads, 2 AV matmuls (emitted
            # one step late).  Fillers per pass feed upcoming passes.
            passes = []
            for p in range(2):
                for xi in range(2):
                    for bl in range(2):
                        passes.append((p, bl, xi))
            # fillers[i] = list of (kt_slot, fn) for pass i
            fillers = [[] for _ in range(8)]
            # V tiles 2..KT-1 as early inserts in pass 1 (deadline: step kt
            # needs v tile kt, inserts run ~1 step after their slot).
            for j, vt in enumerate(range(2, KT)):
                fillers[0].append((j, ("v", vt)))
            # remaining Q chunks, two 4-et halves each, ordered by need:
            # pass2 needs (qc0,bl1); pass3 (qc1,bl0); pass4 (qc1,bl1);
            # pass5 (qc2,bl0); pass6 (qc2,bl1); pass7 (qc3,bl0);
            # pass8 (qc3,bl1).
            qneed = [(1, 0), (1, 1), (2, 0), (2, 1), (3, 0), (3, 1)]
            qslots = [
                (1, 1), (1, 3), (1, 5), (1, 7),
                (2, 1), (2, 3), (2, 5), (2, 7),
                (3, 1), (3, 3), (3, 5), (3, 7),
            ]
            for ci, (qc, bl) in enumerate(qneed):
                for half in range(2):
                    pi, slot = qslots[2 * ci + half]
                    fillers[pi].append((slot, ("q", qc, bl, half)))
            # o_proj: token tiles 0..11 as fillers in pair-1 passes (their
            # vals columns are fully normalized by then); 12..15 in the tail.
            opslots = [
                (4, 1), (4, 4), (4, 7),
                (5, 1), (5, 4), (5, 7),
                (6, 1), (6, 3), (6, 5), (6, 7),
                (7, 4), (7, 8),
            ]
            for j, (pi, slot) in enumerate(opslots):
                fillers[pi].append((slot, ("op", j)))

            def run_filler(spec):
                if spec[0] == "v":
                    emit_v_tile(spec[1], use_opp=(spec[1] % 2 == 1))
                elif spec[0] == "q":
                    emit_q_filler(spec[1], spec[2], spec[3])
                else:
                    emit_op_tile(spec[1])

            pending_norm = None
            norm_state = None
            pending_av = None  # (valsE, valsO, ex, bl, kt)

            def emit_av(valsE, valsO, ex, bl, kt):
                nc.tensor.matmul(
                    valsE,
                    lhsT=v1_sb[:, kt, 2 * bl],
                    rhs=ex[:, 0:512],
                    start=(kt == 0),
                    stop=(kt == KT - 1),
                )
                nc.tensor.matmul(
                    valsO,
                    lhsT=v1_sb[:, kt, 2 * bl + 1],
                    rhs=ex[:, 512:1024],
                    start=(kt == 0),
                    stop=(kt == KT - 1),
                )

            for pi, (p, bl, xi) in enumerate(passes):
                qsrc = qT_sb if p == 0 else qT2_sb
                xs = slice(xi * 512, (xi + 1) * 512)
                pass_fill = sorted(fillers[pi])
                fi = 0
                valsE = valsO = None
                for kt in range(KT):
                    lg = lgp.tile(
                        [128, 1024], f32, tag="lg", name=f"lg_{p}_{bl}_{xi}_{kt}"
                    )
                    ks = slice(kt * 128, (kt + 1) * 128)
                    nc.tensor.matmul(
                        lg[:, 0:512],
                        lhsT=kT_sb[0:64, bl, ks],
                        rhs=qsrc[0:64, bl, xs],
                        start=True,
                        stop=True,
                    )
                    nc.tensor.matmul(
                        lg[:, 512:1024],
                        lhsT=kT_sb[64:128, bl, ks],
                        rhs=qsrc[64:128, bl, xs],
                        start=True,
                        stop=True,
                    )
                    ex = expool.tile(
                        [128, 1024], bf16, tag="ex", name=f"ex_{p}_{bl}_{xi}_{kt}"
                    )
                    nc.scalar.activation(
                        ex, lg, Exp, bias=mb_sb[:, kt : kt + 1], scale=0.125
                    )
                    # flush the previous step's AVs, then (at kt==0) the
                    # previous pass's normalize; vals tiles are allocated
                    # after it so the pool rotation frees banks in
                    # dependency order.
                    if pending_av is not None:
                        emit_av(*pending_av)
                        pending_av = None
                    if kt == 0:
                        if pending_norm is not None:
                            norm_state = emit_norm_pre(*pending_norm)
                            pending_norm = None
                        valsE = valsp.tile(
                            [65, 512], f32, tag="vals", name=f"vals_{p}_{bl}_{xi}_E"
                        )
                        valsO = valsp.tile(
                            [65, 512], f32, tag="vals", name=f"vals_{p}_{bl}_{xi}_O"
                        )
                    if kt == 3 and norm_state is not None:
                        emit_norm_post(norm_state)
                        norm_state = None
                    pending_av = (valsE, valsO, ex, bl, kt)
                    while fi < len(pass_fill) and pass_fill[fi][0] <= kt:
                        run_filler(pass_fill[fi][1])
                        fi += 1
                while fi < len(pass_fill):
                    run_filler(pass_fill[fi][1])
                    fi += 1
                pending_norm = (p, bl, xi, valsE, valsO)

            emit_av(*pending_av)
            pending_av = None
            emit_norm_fast(*pending_norm)

            # ---- o_proj tail: token tiles 12..15.  Nothing left to overlap
            # with, so spread the 8 half-tiles over all 8 freed PSUM banks
            # and drain with both the Scalar and Vector engines.
            def op_homes():
                lga = lgp.tile([128, 1024], f32, tag="lg", name="opfA")
                lgb = lgp.tile([128, 1024], f32, tag="lg", name="opfB")
                yield lga[:, 0:512]
                yield lga[:, 512:1024]
                yield lgb[:, 0:512]
                yield lgb[:, 512:1024]
                for k in range(3):
                    yield valsp.tile([128, 512], f32, tag="vals", name=f"opfv{k}")
                yield opp.tile([128, 512], f32, tag="op", name="opfo")

            homegen = op_homes()
            tail_ops = []
            tc.cur_priority += 1000000  # keep the tail behind all pass work
            # phase 1: valsTa-stage matmuls only -- they depend on the
            # PREVIOUS pass's normalize, so they run (and keep the PE warm)
            # while the final pass's norm chain and vn-DMA are in flight.
            for ttn in range(12, TT):
                for ntn in range(2):
                    op = next(homegen)
                    tail_ops.append((ttn, ntn, op))
                    nc.tensor.matmul(
                        op,
                        lhsT=valsTa[:, ttn * 128 : (ttn + 1) * 128],
                        rhs=wo_sb[:, 0, ntn * 512 : (ntn + 1) * 512],
                        start=True,
                        stop=False,
                    )
            # phase 2: valsTb accumulation + eviction + store per tile
            ots = {}
            for ttn, ntn, op in tail_ops:
                nc.tensor.matmul(
                    op,
                    lhsT=valsTb[:, ttn * 128 : (ttn + 1) * 128],
                    rhs=wo_sb[:, 1, ntn * 512 : (ntn + 1) * 512],
                    start=False,
                    stop=True,
                )
                if ntn == 0:
                    ots[ttn] = outst.tile(
                        [128, 1024], bf16, tag="ot", name=f"otf_{ttn}"
                    )
                    nc.scalar.copy(ots[ttn][:, 0:512], op)
                else:
                    nc.vector.tensor_copy(ots[ttn][:, 512:1024], op)
                    nc.sync.dma_start(
                        out_d[ttn * 128 : (ttn + 1) * 128, :], ots[ttn]
                    )

    nc.compile()
    return nc


def _get_nc(kt_len=KT_LEN):
    if kt_len not in _compiled_nc:
        _compiled_nc[kt_len] = _build_bass(kt_len)
    return _compiled_nc[kt_len]


def pick_kt_len(src_padding_mask):
    """Smallest supported compacted key extent covering every batch's kept
    tokens (KT_LEN default covers it with ~5 sigma of slack for random
    masks; anything larger falls back to a wider, slower build)."""
    need = int(np.max(np.sum(np.asarray(src_padding_mask), axis=1)))
    need = max(need, 256)
    need = (need + 127) // 128 * 128
    return KT_LEN if need <= KT_LEN else need


def make_in_maps(x, src_padding_mask, w_qkv, w_o, kt_len=None):
    """Shard the full inputs into the 8 per-core input maps (all DRAM
    tensors partition-major: [128, ...] with per-partition rows
    contiguous)."""
    import ml_dtypes

    bf16 = ml_dtypes.bfloat16
    if kt_len is None:
        kt_len = pick_kt_len(src_padding_mask)
    x = np.asarray(x, dtype=np.float32)
    mask = np.asarray(src_padding_mask)
    w_qkv = np.asarray(w_qkv, dtype=np.float32)
    w_o = np.asarray(w_o, dtype=np.float32)

    def pmaj(a2d):
        """[E, N] row-major -> [128, ET, N] partition-major."""
        e, n = a2d.shape
        return np.ascontiguousarray(
            a2d.reshape(e // 128, 128, n).transpose(1, 0, 2)
        )

    # w_qkv rows are per-head interleaved: head h -> rows [192h, 192h+192),
    # split 64/64/64 into q/k/v.
    wr = w_qkv.reshape(16, 3, D, E)  # [head, qkv, d, e]

    in_maps = []
    per_batch = {}
    for b in range(B):
        xb = x[b]  # [S, E]
        xT = xb.T  # [E, S]
        # [128, qc, a, t] so each partition's per-q-chunk slab is contiguous
        xpm = (
            xT.reshape(ET, 128, QTILES, 512)
            .transpose(1, 2, 0, 3)
            .reshape(128, -1)
        )
        idx = np.nonzero(mask[b])[0]
        nk = len(idx)
        assert nk <= kt_len, f"kept keys {nk} exceed kt_len {kt_len}"
        xkvT = np.zeros((E, kt_len), np.float32)
        xkvT[:, :nk] = xb[idx].T
        mb = np.full((kt_len,), -30000.0, np.float32)
        mb[:nk] = 0.0
        per_batch[b] = (
            np.ascontiguousarray(xpm).astype(bf16),
            pmaj(xkvT),  # [128, ET, kt_len] f32
            mb,
        )

    for c in range(N_CORES):
        b, g = divmod(c, N_CORES // B)
        xpm, xkvpm, mb = per_batch[b]
        heads = slice(g * HEADS_PER_CORE, (g + 1) * HEADS_PER_CORE)
        wq = wr[heads, 0].reshape(256, E)  # [4*64, E]
        wk = wr[heads, 1].reshape(256, E)
        wv = wr[heads, 2].reshape(256, E)
        wo = (
            w_o[:, g * 256 : (g + 1) * 256]
            .reshape(E, 2, 2, D)[:, :, ::-1, :]
            .reshape(E, 256)
            .T
        )  # [256, E]
        # fuse wk and xkv along the per-contraction-tile free axis
        kx = np.concatenate([pmaj(wk.T), xkvpm], axis=2)  # [128, ET, 256+kt]
        in_maps.append(
            {
                "xp": xpm,
                "kxp": np.ascontiguousarray(kx).reshape(128, -1).astype(bf16),
                "wqp": pmaj(wq.T).reshape(128, -1).astype(bf16),
                "wvp": pmaj(wv.T).reshape(128, -1).astype(bf16),
                "wop": pmaj(wo).reshape(128, -1).astype(bf16),
                "mbias": mb,
            }
        )
    return in_maps


def combine_outputs(outs):
    """Sum the 4 per-head-group partials for each batch."""
    full = np.zeros((B, S, E), np.float32)
    for c in range(N_CORES):
        full[c // (N_CORES // B)] += np.asarray(outs[c]).astype(np.float32)
    return full


def kernel(x, src_padding_mask, w_qkv, w_o, _trace=False):
    from concourse.bass_utils import run_bass_kernel_spmd

    kt_len = pick_kt_len(src_padding_mask)
    nc = _get_nc(kt_len)
    in_maps = make_in_maps(x, src_padding_mask, w_qkv, w_o, kt_len)
    kwargs = {}
    if _trace:
        kwargs = dict(trace=True, trace_cores=list(range(N_CORES)))
    res = run_bass_kernel_spmd(nc, in_maps, core_ids=list(range(N_CORES)), **kwargs)
    out = combine_outputs([r["out"] for r in res.results])
    if _trace:
        kernel._last_result = res
    return out
